# revision 9
# baseline (speedup 1.0000x reference)
"""Causal attention head (B=4, S=4096, D=512, E=64) on 8 TRN2 NeuronCores.

Sharding: per batch b, core pair (2b, 2b+1); zig-zag query blocks
(QSTARTS) balance causal work; uniform SLOT_J keeps the program SPMD.

v3 streaming pipeline:
 - All input loads issued upfront on the sync HWDGE ring, ordered by
   first-consumer deadline (xq0, xk0, masksA, xk1, xv0, ...), so the
   ring is never blocked by compute-dependent DMAs.
 - V is projected as V^T (cheap N=512 matmuls) into vt (rows 64:80 are
   ones for the softmax denominator), then moved to [keys, E+1] chunk
   layout via SBUF->SBUF DMA-transpose issued from the *scalar* HWDGE
   ring at points where the dependency is already resolved.
 - Deferred-PV software pipeline: slot s's mask-muls + PV matmuls are
   emitted inside slot s+1's pair loop (pairs 4+), so the in-order PE
   stream never stalls the score/exp stream on V availability.  Slot 3
   uses the classic prev-pair inline pipeline for its own PVs.
 - Projection matmul groups of later segments are stuffed at pairs
   chosen to match DMA arrival times; K/Q use a [w|w] duplicated
   stationary so the partition-packed score matmuls get their
   duplicated operands for free.
 - ScalarE runs exp only (plus 4 cheap transpose-DMA issues); all
   PSUM->SBUF copies are on VectorE; z output rows are batched per slot
   and written via SWDGE (gpsimd).
 - A short burst of dummy matmuls right after the weight DMAs lifts the
   PE out of the HAM 1.2 GHz cold state before the real projections.
"""

import sys

sys.path.insert(0, "/opt/trn_rl_repo")

import numpy as np
import ml_dtypes

from concourse import bacc, mybir
from concourse import tile
from concourse.bass_utils import run_bass_kernel_spmd

BF16 = ml_dtypes.bfloat16
F32 = mybir.dt.float32
BF = mybir.dt.bfloat16

B, S, D, E = 4, 4096, 512, 64
P = 128
NQ = 2048          # queries per core
QBLK = 512         # query block
NCH = D // P       # 4 contraction chunks for projections
NKCH = S // P      # 32 key chunks
NSEG = 4           # 1024-key segments
QSTARTS = {0: [0, 1024, 2048, 3072], 1: [512, 1536, 2560, 3584]}
SLOT_J = [8, 16, 24, 32]  # uniform per-slot key-chunk counts (all cores)

_CACHE = {}
LAST_RESULT = None


def _build():
    nc = bacc.Bacc(
        "TRN2",
        target_bir_lowering=False,
        debug=False,
        enable_asserts=True,
        num_devices=8,
    )

    xqt_d = nc.declare_dram_parameter("xqt", [D, NQ], BF, isOutput=False)
    xkt_d = nc.declare_dram_parameter("xkt", [D, S], BF, isOutput=False)
    xvt_d = nc.declare_dram_parameter("xvt", [D, S], BF, isOutput=False)
    wq = nc.declare_dram_parameter("wq", [D, E], BF, isOutput=False)  # pre-scaled 1/8
    wk = nc.declare_dram_parameter("wk", [D, E], BF, isOutput=False)
    wv = nc.declare_dram_parameter("wv", [D, E], BF, isOutput=False)
    masks = nc.declare_dram_parameter("masks", [P, 8 * QBLK], BF, isOutput=False)
    ident = nc.declare_dram_parameter("ident", [P, P], F32, isOutput=False)
    zout = nc.declare_dram_parameter("z", [NQ, E], F32, isOutput=True)

    with tile.TileContext(nc) as tc:
        with (
            tc.tile_pool(name="const", bufs=1) as const,
            tc.tile_pool(name="xt", bufs=1) as xt,
            tc.tile_pool(name="proj", bufs=1) as proj,
            tc.tile_pool(name="work", bufs=16) as work,
            tc.tile_pool(name="epi", bufs=2) as epi,
            tc.tile_pool(name="psA", bufs=2, space="PSUM") as psA,
            tc.tile_pool(name="psZ", bufs=2, space="PSUM") as psZ,
            tc.tile_pool(name="psB", bufs=2, space="PSUM") as psB,
        ):
            # ---- ACT table warmup: tiny exp as early as possible ----
            warm = const.tile([1, 16], F32, tag="warm")
            warm2 = const.tile([1, 16], F32, tag="warm2")
            nc.gpsimd.memset(warm, 0.0)
            nc.scalar.activation(
                out=warm2, in_=warm, func=mybir.ActivationFunctionType.Exp
            )

            # ---- constants (duplicated wq/wk for partition-packed scores) ----
            wq_sb = const.tile([P, NCH, P], BF, tag="wq")
            wk_sb = const.tile([P, NCH, P], BF, tag="wk")
            wv_sb = const.tile([P, NCH, E], BF, tag="wv")
            for w_dram, w_sb in ((wq, wq_sb), (wk, wk_sb)):
                for half in range(2):
                    nc.sync.dma_start(
                        out=w_sb[:, :, half * E : (half + 1) * E],
                        in_=w_dram.rearrange("(c p) e -> p c e", p=P),
                    )
            nc.sync.dma_start(
                out=wv_sb[:, :, :], in_=wv.rearrange("(c p) e -> p c e", p=P)
            )
            ident_sb = const.tile([P, P], F32, tag="ident")
            nc.sync.dma_start(out=ident_sb[:, :], in_=ident[:, :])

            # ---- X tiles, per 1024-col segment ----
            xq_s = [
                xt.tile([P, NCH, 1024], BF, tag=f"xq{s}", name=f"xq_s{s}")
                for s in range(2)
            ]
            xk_s = [
                xt.tile([P, NCH, 1024], BF, tag=f"xk{s}", name=f"xk_s{s}")
                for s in range(NSEG)
            ]
            xv_s = [
                xt.tile([P, NCH, 1024], BF, tag=f"xv{s}", name=f"xv_s{s}")
                for s in range(NSEG)
            ]
            masks_sb = const.tile([P, 8 * QBLK], BF, tag="masks")

            def load_x(dst, src_d, s):
                nc.sync.dma_start(
                    out=dst[:, :, :],
                    in_=src_d[:, s * 1024 : (s + 1) * 1024].rearrange(
                        "(c p) r -> p c r", p=P
                    ),
                )

            # all input loads upfront, in first-consumer order
            load_x(xq_s[0], xqt_d, 0)
            load_x(xk_s[0], xkt_d, 0)
            nc.sync.dma_start(out=masks_sb[:, 0:2048], in_=masks[:, 0:2048])
            load_x(xk_s[1], xkt_d, 1)
            load_x(xv_s[0], xvt_d, 0)
            nc.sync.dma_start(out=masks_sb[:, 2048:4096], in_=masks[:, 2048:4096])
            nc.sync.dma_start(  # xq seg1 first half (query block 2)
                out=xq_s[1][:, :, 0:512],
                in_=xqt_d[:, 1024:1536].rearrange("(c p) r -> p c r", p=P),
            )
            load_x(xk_s[2], xkt_d, 2)
            load_x(xv_s[1], xvt_d, 1)
            nc.sync.dma_start(  # xq seg1 second half (query block 3)
                out=xq_s[1][:, :, 512:1024],
                in_=xqt_d[:, 1536:2048].rearrange("(c p) r -> p c r", p=P),
            )
            load_x(xk_s[3], xkt_d, 3)
            load_x(xv_s[2], xvt_d, 2)
            load_x(xv_s[3], xvt_d, 3)

            # ---- projection outputs ----
            qt2 = proj.tile([P, NQ], BF, tag="qt")
            kt2 = proj.tile([P, S], BF, tag="kt")
            vt = proj.tile([80, S], BF, tag="vt")       # rows 0:64 V^T, 64:80 ones
            vp = proj.tile([P, NKCH, 80], BF, tag="vp")  # [keys, chunk, E+ones]
            nc.gpsimd.memset(vt[64:80, :], 1.0)

            # ---- PE HAM warmup: dummy matmuls on a memset tile (no DMA dep,
            # runs right after iram load so the PE is at 2.4 GHz when real
            # projections start) ----
            wmm = const.tile([P, P], BF, tag="wmm")
            nc.gpsimd.memset(wmm, 0.0)
            wps = psA.tile([P, P], F32, tag="st")
            for r in range(40):
                nc.tensor.matmul(
                    wps,
                    lhsT=wmm,
                    rhs=wmm,
                    start=(r == 0),
                    stop=(r == 39),
                )
            wsink = work.tile([P, P], BF, tag="pt", name="wsink")
            nc.vector.tensor_copy(wsink, wps)

            def proj_group_kq(w_sb, x_tile, dst, seg_local_g, dst_col):
                ps = psB.tile([P, QBLK], F32, tag="pj")
                for c in range(NCH):
                    nc.tensor.matmul(
                        ps,
                        lhsT=w_sb[:, c, :],
                        rhs=x_tile[:, c, seg_local_g * QBLK : (seg_local_g + 1) * QBLK],
                        start=(c == 0),
                        stop=(c == NCH - 1),
                    )
                nc.vector.tensor_copy(dst[:, dst_col : dst_col + QBLK], ps)

            def proj_group_v(seg, seg_local_g):
                ps = psB.tile([E, QBLK], F32, tag="pj")
                for c in range(NCH):
                    nc.tensor.matmul(
                        ps,
                        lhsT=wv_sb[:, c, :],
                        rhs=xv_s[seg][:, c, seg_local_g * QBLK : (seg_local_g + 1) * QBLK],
                        start=(c == 0),
                        stop=(c == NCH - 1),
                    )
                col = seg * 1024 + seg_local_g * QBLK
                nc.vector.tensor_copy(vt[0:E, col : col + QBLK], ps)

            def transpose_v(seg):
                # sync HWDGE ring (idle once the upfront loads are issued);
                # vt --xbar--> vp[p][chunk][e] = V[c*128+p][e]
                nc.sync.dma_start(
                    out=vp[:, seg * 8 : (seg + 1) * 8, :],
                    in_=vt[:, seg * 1024 : (seg + 1) * 1024],
                    transpose=True,
                )

            def k_group(seg, g):
                return lambda: proj_group_kq(
                    wk_sb, xk_s[seg], kt2, g, seg * 1024 + g * QBLK
                )

            def q_group(seg, g):
                return lambda: proj_group_kq(
                    wq_sb, xq_s[seg], qt2, g, seg * 1024 + g * QBLK
                )

            def v_group(seg, g):
                return lambda: proj_group_v(seg, g)

            # upfront projections (data arrives ~8-12us)
            proj_group_kq(wq_sb, xq_s[0], qt2, 0, 0)
            proj_group_kq(wq_sb, xq_s[0], qt2, 1, QBLK)
            proj_group_kq(wk_sb, xk_s[0], kt2, 0, 0)
            proj_group_kq(wk_sb, xk_s[0], kt2, 1, QBLK)

            # per-slot stuffing of projection groups (keyed by pair index),
            # and per-slot transpose-issue points (after exp of that pair)
            stuffing = {
                0: {2: k_group(1, 0), 3: k_group(1, 1)},
                1: {2: v_group(0, 0), 3: v_group(0, 1),
                    5: q_group(1, 0), 7: k_group(2, 0)},
                2: {0: v_group(1, 0), 1: v_group(1, 1), 2: k_group(2, 1),
                    4: q_group(1, 1), 6: k_group(3, 0), 8: k_group(3, 1),
                    10: v_group(2, 0), 11: v_group(2, 1)},
                3: {0: v_group(3, 0), 1: v_group(3, 1)},
            }
            tissue = {0: {}, 1: {4: 0}, 2: {3: 1}, 3: {0: 2, 3: 3}}

            # ---- attention ----
            def emit_epilogue(zps, ib):
                zsb = epi.tile([E + 1, QBLK], F32, tag="zsb")
                nc.vector.tensor_copy(zsb, zps)
                zf = epi.tile([P, QBLK // P, E], F32, tag="zf")
                for u in range(QBLK // P):
                    zbp = psB.tile([P, E + 1], F32, tag="pj")
                    nc.tensor.transpose(
                        zbp,
                        zsb[:, u * P : (u + 1) * P],
                        ident_sb[0 : E + 1, 0 : E + 1],
                    )
                    rc = epi.tile([P, 1], F32, tag="rc")
                    nc.vector.reciprocal(rc, zbp[:, E : E + 1])
                    nc.vector.tensor_scalar_mul(zf[:, u, :], zbp[:, 0:E], rc)
                nc.gpsimd.dma_start(
                    out=zout[ib * QBLK : (ib + 1) * QBLK, :].rearrange(
                        "(u p) e -> p u e", p=P
                    ),
                    in_=zf[:, :, :],
                )

            def mask_mul(pt, jmax, jp):
                j0 = 2 * jp
                if j0 >= jmax - 8:
                    m = j0 - (jmax - 8)
                    nc.vector.tensor_mul(
                        pt, pt, masks_sb[:, m * QBLK : (m + 2) * QBLK]
                    )

            def emit_pv(zps, jmax, pt, jp):
                for h in range(2):
                    j = 2 * jp + h
                    nc.tensor.matmul(
                        zps,
                        lhsT=vp[:, j, 0 : E + 1],
                        rhs=pt[:, h * QBLK : (h + 1) * QBLK],
                        start=(j == 0),
                        stop=(j == jmax - 1),
                        skip_group_check=True,
                    )

            # deferred records from the previous slot:
            # dict(zps, jmax, ib, pairs=[(pt, jp), ...])
            deferred = None

            for ib in range(4):
                jmax = SLOT_J[ib]
                qloc = ib * QBLK
                zps = psZ.tile([E + 1, QBLK], F32, tag="zt")
                stuff = stuffing[ib]
                tiss = tissue[ib]
                new_pairs = []
                prev = None  # slot3 inline pipeline
                drain = list(deferred["pairs"]) if deferred else []

                for jp in range(jmax // 2):
                    sps = psA.tile([P, 2 * QBLK], F32, tag="st")
                    for h in range(2):
                        j = 2 * jp + h
                        nc.tensor.matmul(
                            sps[:, h * QBLK : (h + 1) * QBLK],
                            lhsT=kt2[h * E : (h + 1) * E, j * P : (j + 1) * P],
                            rhs=qt2[h * E : (h + 1) * E, qloc : qloc + QBLK],
                            start=True,
                            stop=True,
                            tile_position=(h * E, 0),
                        )
                    pt = work.tile([P, 2 * QBLK], BF, tag="pt")
                    nc.scalar.activation(
                        out=pt, in_=sps, func=mybir.ActivationFunctionType.Exp
                    )
                    if jp in tiss:
                        transpose_v(tiss[jp])
                    if ib == 3:
                        mask_mul(pt, jmax, jp)
                        if prev is not None:
                            emit_pv(zps, jmax, *prev)
                        prev = (pt, jp)
                    if jp >= 4 and drain:
                        dpt, djp = drain.pop(0)
                        mask_mul(dpt, deferred["jmax"], djp)
                        emit_pv(deferred["zps"], deferred["jmax"], dpt, djp)
                        if not drain:
                            emit_epilogue(deferred["zps"], deferred["ib"])
                    if jp in stuff:
                        stuff[jp]()
                    if ib < 3:
                        new_pairs.append((pt, jp))

                if ib == 3:
                    emit_pv(zps, jmax, *prev)
                    # drain any leftovers (shouldn't happen: 12 fit in p4-15)
                    while drain:
                        dpt, djp = drain.pop(0)
                        mask_mul(dpt, deferred["jmax"], djp)
                        emit_pv(deferred["zps"], deferred["jmax"], dpt, djp)
                        if not drain:
                            emit_epilogue(deferred["zps"], deferred["ib"])
                    emit_epilogue(zps, ib)
                else:
                    deferred = {
                        "zps": zps,
                        "jmax": jmax,
                        "ib": ib,
                        "pairs": new_pairs,
                    }

    nc.compile()
    return nc


def _get_nc():
    if "nc" not in _CACHE:
        _CACHE["nc"] = _build()
    return _CACHE["nc"]


def _ensure_ntff_hook():
    """Install antenv.axon_hooks + NTFF profile hook if the image lacks it."""
    import types

    try:
        from antenv import axon_hooks  # noqa: F401

        return
    except ImportError:
        pass
    import antenv
    from concourse import bass_utils as _bu

    mod = types.ModuleType("antenv.axon_hooks")
    _state = {}
    mod.set_axon_ntff_profile_hook = lambda h: _state.__setitem__("h", h)
    mod.get_axon_ntff_profile_hook = lambda: _state.get("h")
    sys.modules["antenv.axon_hooks"] = mod
    antenv.axon_hooks = mod
    sys.path.insert(0, "/root/.axon_site/trn_agent_boot")
    from trn_boot import _ntff_profile_via_ctypes

    mod.set_axon_ntff_profile_hook(
        _ntff_profile_via_ctypes("/opt/axon/libaxon_pjrt.so")
    )
    _bu.upload_artifacts = lambda tmpdir: f"local://{tmpdir}"


def _make_masks(h):
    kl = np.arange(P)[:, None]
    ql = np.arange(QBLK)[None, :]
    diag = [(kl <= ql - P * t).astype(np.float32) for t in range(4)]
    ones = np.ones((P, QBLK), np.float32)
    zero = np.zeros((P, QBLK), np.float32)
    tiles = diag + [zero] * 4 if h == 0 else [ones] * 4 + diag
    return np.concatenate(tiles, axis=1).astype(BF16)


def kernel(key_inputs, value_inputs, query_inputs, Wq, Wk, Wv):
    global LAST_RESULT
    import os

    key_inputs = np.asarray(key_inputs, dtype=np.float32)
    value_inputs = np.asarray(value_inputs, dtype=np.float32)
    query_inputs = np.asarray(query_inputs, dtype=np.float32)
    wq_b = (np.asarray(Wq, dtype=np.float32) * 0.125).astype(BF16)
    wk_b = np.asarray(Wk, dtype=np.float32).astype(BF16)
    wv_b = np.asarray(Wv, dtype=np.float32).astype(BF16)
    masks_np = [_make_masks(0), _make_masks(1)]
    ident_np = np.eye(P, dtype=np.float32)

    in_maps = []
    for c in range(8):
        b, h = c // 2, c % 2
        xq_c = np.concatenate(
            [query_inputs[b, q0 : q0 + QBLK] for q0 in QSTARTS[h]], axis=0
        )
        xk_c = key_inputs[b]
        xv_c = value_inputs[b]
        in_maps.append(
            {
                "xqt": np.ascontiguousarray(xq_c.T).astype(BF16),
                "xkt": np.ascontiguousarray(xk_c.T).astype(BF16),
                "xvt": np.ascontiguousarray(xv_c.T).astype(BF16),
                "wq": wq_b,
                "wk": wk_b,
                "wv": wv_b,
                "masks": masks_np[h],
                "ident": ident_np,
            }
        )

    nc = _get_nc()
    trace = bool(int(os.environ.get("KERNEL_TRACE", "0")))
    if trace:
        _ensure_ntff_hook()
    res = run_bass_kernel_spmd(
        nc,
        in_maps,
        core_ids=list(range(8)),
        trace=trace,
        tmpdir=os.environ.get("KERNEL_TRACE_DIR") or None,
    )
    LAST_RESULT = res

    out = np.empty((B, S, E), dtype=np.float32)
    for c in range(8):
        b, h = c // 2, c % 2
        z = np.asarray(res.results[c]["z"], dtype=np.float32)
        for ib, q0 in enumerate(QSTARTS[h]):
            out[b, q0 : q0 + QBLK] = z[ib * QBLK : (ib + 1) * QBLK]
    return out


# revision 15
# speedup vs baseline: 1.0021x; 1.0021x over previous
"""Causal attention head (B=4, S=4096, D=512, E=64) on 8 TRN2 NeuronCores.

Sharding: per batch b, core pair (2b, 2b+1); zig-zag query blocks
(QSTARTS) balance causal work; uniform SLOT_J keeps the program SPMD.

v3 streaming pipeline:
 - All input loads issued upfront on the sync HWDGE ring, ordered by
   first-consumer deadline (xq0, xk0, masksA, xk1, xv0, ...), so the
   ring is never blocked by compute-dependent DMAs.
 - V is projected as V^T (cheap N=512 matmuls) into vt (rows 64:80 are
   ones for the softmax denominator), then moved to [keys, E+1] chunk
   layout via SBUF->SBUF DMA-transpose issued from the *scalar* HWDGE
   ring at points where the dependency is already resolved.
 - Deferred-PV software pipeline: slot s's mask-muls + PV matmuls are
   emitted inside slot s+1's pair loop (pairs 4+), so the in-order PE
   stream never stalls the score/exp stream on V availability.  Slot 3
   uses the classic prev-pair inline pipeline for its own PVs.
 - Projection matmul groups of later segments are stuffed at pairs
   chosen to match DMA arrival times; K/Q use a [w|w] duplicated
   stationary so the partition-packed score matmuls get their
   duplicated operands for free.
 - ScalarE runs exp only (plus 4 cheap transpose-DMA issues); all
   PSUM->SBUF copies are on VectorE; z output rows are batched per slot
   and written via SWDGE (gpsimd).
 - A short burst of dummy matmuls right after the weight DMAs lifts the
   PE out of the HAM 1.2 GHz cold state before the real projections.
"""

import sys

sys.path.insert(0, "/opt/trn_rl_repo")

import numpy as np
import ml_dtypes

from concourse import bacc, mybir
from concourse import tile
from concourse.bass_utils import run_bass_kernel_spmd

BF16 = ml_dtypes.bfloat16
F32 = mybir.dt.float32
BF = mybir.dt.bfloat16

B, S, D, E = 4, 4096, 512, 64
P = 128
NQ = 2048          # queries per core
QBLK = 512         # query block
NCH = D // P       # 4 contraction chunks for projections
NKCH = S // P      # 32 key chunks
NSEG = 4           # 1024-key segments
QSTARTS = {0: [0, 1024, 2048, 3072], 1: [512, 1536, 2560, 3584]}
SLOT_J = [8, 16, 24, 32]  # uniform per-slot key-chunk counts (all cores)

_CACHE = {}
LAST_RESULT = None


def _build():
    nc = bacc.Bacc(
        "TRN2",
        target_bir_lowering=False,
        debug=False,
        enable_asserts=True,
        num_devices=8,
    )

    xqt_d = nc.declare_dram_parameter("xqt", [D, NQ], BF, isOutput=False)
    xkt_d = nc.declare_dram_parameter("xkt", [D, S], BF, isOutput=False)
    xvt_d = nc.declare_dram_parameter("xvt", [D, S], BF, isOutput=False)
    wq = nc.declare_dram_parameter("wq", [D, E], BF, isOutput=False)  # pre-scaled 1/8
    wk = nc.declare_dram_parameter("wk", [D, E], BF, isOutput=False)
    wv = nc.declare_dram_parameter("wv", [D, E], BF, isOutput=False)
    masks = nc.declare_dram_parameter(
        "masks", [P, 8 * QBLK], mybir.dt.float8e4, isOutput=False
    )
    ident = nc.declare_dram_parameter("ident", [P, P], F32, isOutput=False)
    zout = nc.declare_dram_parameter("z", [NQ, E], F32, isOutput=True)

    with tile.TileContext(nc) as tc:
        with (
            tc.tile_pool(name="const", bufs=1) as const,
            tc.tile_pool(name="xt", bufs=1) as xt,
            tc.tile_pool(name="proj", bufs=1) as proj,
            tc.tile_pool(name="work", bufs=16) as work,
            tc.tile_pool(name="epi", bufs=2) as epi,
            tc.tile_pool(name="psA", bufs=2, space="PSUM") as psA,
            tc.tile_pool(name="psZ", bufs=2, space="PSUM") as psZ,
            tc.tile_pool(name="psB", bufs=2, space="PSUM") as psB,
        ):
            # ---- ACT table warmup: tiny exp as early as possible ----
            warm = const.tile([1, 16], F32, tag="warm")
            warm2 = const.tile([1, 16], F32, tag="warm2")
            nc.gpsimd.memset(warm, 0.0)
            nc.scalar.activation(
                out=warm2, in_=warm, func=mybir.ActivationFunctionType.Exp
            )

            # ---- constants (duplicated wq/wk for partition-packed scores) ----
            wq_sb = const.tile([P, NCH, P], BF, tag="wq")
            wk_sb = const.tile([P, NCH, P], BF, tag="wk")
            wv_sb = const.tile([P, NCH, E], BF, tag="wv")
            for w_dram, w_sb in ((wq, wq_sb), (wk, wk_sb)):
                for half in range(2):
                    nc.sync.dma_start(
                        out=w_sb[:, :, half * E : (half + 1) * E],
                        in_=w_dram.rearrange("(c p) e -> p c e", p=P),
                    )
            nc.sync.dma_start(
                out=wv_sb[:, :, :], in_=wv.rearrange("(c p) e -> p c e", p=P)
            )
            ident_sb = const.tile([P, P], F32, tag="ident")
            nc.sync.dma_start(out=ident_sb[:, :], in_=ident[:, :])

            # ---- X tiles, per 1024-col segment ----
            xq_s = [
                xt.tile([P, NCH, 1024], BF, tag=f"xq{s}", name=f"xq_s{s}")
                for s in range(2)
            ]
            xk_s = [
                xt.tile([P, NCH, 1024], BF, tag=f"xk{s}", name=f"xk_s{s}")
                for s in range(NSEG)
            ]
            xv_s = [
                xt.tile([P, NCH, 1024], BF, tag=f"xv{s}", name=f"xv_s{s}")
                for s in range(NSEG)
            ]
            masks_sb = const.tile([P, 8 * QBLK], BF, tag="masks")

            def load_x(dst, src_d, s):
                nc.sync.dma_start(
                    out=dst[:, :, :],
                    in_=src_d[:, s * 1024 : (s + 1) * 1024].rearrange(
                        "(c p) r -> p c r", p=P
                    ),
                )

            # masks via SWDGE cast-DMA (fp8 -> bf16) on the idle gpsimd ring,
            # keeping the sync ring free for the big input loads
            nc.gpsimd.dma_start(out=masks_sb[:, 0:2048], in_=masks[:, 0:2048])
            nc.gpsimd.dma_start(out=masks_sb[:, 2048:4096], in_=masks[:, 2048:4096])

            # all input loads upfront, in first-consumer order
            load_x(xq_s[0], xqt_d, 0)
            load_x(xk_s[0], xkt_d, 0)
            load_x(xv_s[0], xvt_d, 0)
            load_x(xk_s[1], xkt_d, 1)
            nc.sync.dma_start(  # xq seg1 first half (query block 2)
                out=xq_s[1][:, :, 0:512],
                in_=xqt_d[:, 1024:1536].rearrange("(c p) r -> p c r", p=P),
            )
            load_x(xk_s[2], xkt_d, 2)
            load_x(xv_s[1], xvt_d, 1)
            nc.sync.dma_start(  # xq seg1 second half (query block 3)
                out=xq_s[1][:, :, 512:1024],
                in_=xqt_d[:, 1536:2048].rearrange("(c p) r -> p c r", p=P),
            )
            load_x(xk_s[3], xkt_d, 3)
            load_x(xv_s[2], xvt_d, 2)
            load_x(xv_s[3], xvt_d, 3)

            # ---- projection outputs ----
            qt2 = proj.tile([P, NQ], BF, tag="qt")
            kt2 = proj.tile([P, S], BF, tag="kt")
            vt = proj.tile([80, S], BF, tag="vt")       # rows 0:64 V^T, 64:80 ones
            vp = proj.tile([P, NKCH, 80], BF, tag="vp")  # [keys, chunk, E+ones]
            nc.gpsimd.memset(vt[64:80, :], 1.0)

            # ---- PE HAM warmup: dummy matmuls on a memset tile (no DMA dep,
            # runs right after iram load so the PE is at 2.4 GHz when real
            # projections start) ----
            wmm = const.tile([P, P], BF, tag="wmm")
            nc.gpsimd.memset(wmm, 0.0)
            wps = psA.tile([P, P], F32, tag="st")
            for r in range(40):
                nc.tensor.matmul(
                    wps,
                    lhsT=wmm,
                    rhs=wmm,
                    start=(r == 0),
                    stop=(r == 39),
                )
            wsink = work.tile([P, P], BF, tag="pt", name="wsink")
            nc.vector.tensor_copy(wsink, wps)

            def proj_group_kq(w_sb, x_tile, dst, seg_local_g, dst_col):
                ps = psB.tile([P, QBLK], F32, tag="pj")
                for c in range(NCH):
                    nc.tensor.matmul(
                        ps,
                        lhsT=w_sb[:, c, :],
                        rhs=x_tile[:, c, seg_local_g * QBLK : (seg_local_g + 1) * QBLK],
                        start=(c == 0),
                        stop=(c == NCH - 1),
                    )
                nc.vector.tensor_copy(dst[:, dst_col : dst_col + QBLK], ps)

            def proj_group_v(seg, seg_local_g):
                ps = psB.tile([E, QBLK], F32, tag="pj")
                for c in range(NCH):
                    nc.tensor.matmul(
                        ps,
                        lhsT=wv_sb[:, c, :],
                        rhs=xv_s[seg][:, c, seg_local_g * QBLK : (seg_local_g + 1) * QBLK],
                        start=(c == 0),
                        stop=(c == NCH - 1),
                    )
                col = seg * 1024 + seg_local_g * QBLK
                nc.vector.tensor_copy(vt[0:E, col : col + QBLK], ps)

            def transpose_v(seg):
                # scalar HWDGE ring: transfers are not queued behind the input
                # loads (sync ring FIFO); issue points are placed where the
                # vt casts are already complete so ACT never blocks.
                # vt --xbar--> vp[p][chunk][e] = V[c*128+p][e]
                nc.scalar.dma_start(
                    out=vp[:, seg * 8 : (seg + 1) * 8, :],
                    in_=vt[:, seg * 1024 : (seg + 1) * 1024],
                    transpose=True,
                )

            def k_group(seg, g):
                return lambda: proj_group_kq(
                    wk_sb, xk_s[seg], kt2, g, seg * 1024 + g * QBLK
                )

            def q_group(seg, g):
                return lambda: proj_group_kq(
                    wq_sb, xq_s[seg], qt2, g, seg * 1024 + g * QBLK
                )

            def v_group(seg, g):
                return lambda: proj_group_v(seg, g)

            # upfront projections (data arrives ~8-12us)
            proj_group_kq(wq_sb, xq_s[0], qt2, 0, 0)
            proj_group_kq(wq_sb, xq_s[0], qt2, 1, QBLK)
            proj_group_kq(wk_sb, xk_s[0], kt2, 0, 0)
            proj_group_kq(wk_sb, xk_s[0], kt2, 1, QBLK)

            # per-slot stuffing of projection groups (keyed by pair index),
            # and per-slot transpose-issue points (after exp of that pair)
            stuffing = {
                0: {2: k_group(1, 0), 3: k_group(1, 1)},
                1: {2: v_group(0, 0), 3: v_group(0, 1),
                    5: q_group(1, 0), 7: k_group(2, 0)},
                2: {0: v_group(1, 0), 1: v_group(1, 1), 2: k_group(2, 1),
                    4: q_group(1, 1), 6: v_group(2, 0), 7: v_group(2, 1),
                    8: k_group(3, 0), 9: k_group(3, 1),
                    10: v_group(3, 0), 11: v_group(3, 1)},
                3: {},
            }
            tissue = {0: {}, 1: {4: 0}, 2: {2: 1, 9: 2}, 3: {2: 3}}

            # ---- attention ----
            def emit_epilogue(zps, ib):
                zsb = epi.tile([E + 1, QBLK], F32, tag="zsb")
                nc.vector.tensor_copy(zsb, zps)
                zf = epi.tile([P, QBLK // P, E], F32, tag="zf")
                for u in range(QBLK // P):
                    zbp = psB.tile([P, E + 1], F32, tag="pj")
                    nc.tensor.transpose(
                        zbp,
                        zsb[:, u * P : (u + 1) * P],
                        ident_sb[0 : E + 1, 0 : E + 1],
                    )
                    rc = epi.tile([P, 1], F32, tag="rc")
                    nc.vector.reciprocal(rc, zbp[:, E : E + 1])
                    nc.vector.tensor_scalar_mul(zf[:, u, :], zbp[:, 0:E], rc)
                nc.gpsimd.dma_start(
                    out=zout[ib * QBLK : (ib + 1) * QBLK, :].rearrange(
                        "(u p) e -> p u e", p=P
                    ),
                    in_=zf[:, :, :],
                )

            def mask_mul(pt, jmax, jp):
                j0 = 2 * jp
                if j0 >= jmax - 8:
                    m = j0 - (jmax - 8)
                    nc.vector.tensor_mul(
                        pt, pt, masks_sb[:, m * QBLK : (m + 2) * QBLK]
                    )

            def emit_pv(zps, jmax, pt, jp):
                for h in range(2):
                    j = 2 * jp + h
                    nc.tensor.matmul(
                        zps,
                        lhsT=vp[:, j, 0 : E + 1],
                        rhs=pt[:, h * QBLK : (h + 1) * QBLK],
                        start=(j == 0),
                        stop=(j == jmax - 1),
                        skip_group_check=True,
                    )

            # deferred records from the previous slot:
            # dict(zps, jmax, ib, pairs=[(pt, jp), ...])
            deferred = None

            for ib in range(4):
                jmax = SLOT_J[ib]
                qloc = ib * QBLK
                zps = psZ.tile([E + 1, QBLK], F32, tag="zt")
                stuff = stuffing[ib]
                tiss = tissue[ib]
                new_pairs = []
                prev = None  # slot3 inline pipeline
                drain = list(deferred["pairs"]) if deferred else []

                for jp in range(jmax // 2):
                    sps = psA.tile([P, 2 * QBLK], F32, tag="st")
                    for h in range(2):
                        j = 2 * jp + h
                        nc.tensor.matmul(
                            sps[:, h * QBLK : (h + 1) * QBLK],
                            lhsT=kt2[h * E : (h + 1) * E, j * P : (j + 1) * P],
                            rhs=qt2[h * E : (h + 1) * E, qloc : qloc + QBLK],
                            start=True,
                            stop=True,
                            tile_position=(h * E, 0),
                        )
                    pt = work.tile([P, 2 * QBLK], BF, tag="pt")
                    nc.scalar.activation(
                        out=pt, in_=sps, func=mybir.ActivationFunctionType.Exp
                    )
                    if jp in tiss:
                        transpose_v(tiss[jp])
                    if ib == 3:
                        mask_mul(pt, jmax, jp)
                        if prev is not None:
                            emit_pv(zps, jmax, *prev)
                        prev = (pt, jp)
                    if jp >= 4 and drain:
                        dpt, djp = drain.pop(0)
                        mask_mul(dpt, deferred["jmax"], djp)
                        emit_pv(deferred["zps"], deferred["jmax"], dpt, djp)
                        if not drain:
                            emit_epilogue(deferred["zps"], deferred["ib"])
                    if jp in stuff:
                        stuff[jp]()
                    if ib < 3:
                        new_pairs.append((pt, jp))

                if ib == 3:
                    emit_pv(zps, jmax, *prev)
                    # drain any leftovers (shouldn't happen: 12 fit in p4-15)
                    while drain:
                        dpt, djp = drain.pop(0)
                        mask_mul(dpt, deferred["jmax"], djp)
                        emit_pv(deferred["zps"], deferred["jmax"], dpt, djp)
                        if not drain:
                            emit_epilogue(deferred["zps"], deferred["ib"])
                    emit_epilogue(zps, ib)
                else:
                    deferred = {
                        "zps": zps,
                        "jmax": jmax,
                        "ib": ib,
                        "pairs": new_pairs,
                    }

    nc.compile()
    return nc


def _get_nc():
    if "nc" not in _CACHE:
        _CACHE["nc"] = _build()
    return _CACHE["nc"]


def _ensure_ntff_hook():
    """Install antenv.axon_hooks + NTFF profile hook if the image lacks it."""
    import types

    try:
        from antenv import axon_hooks  # noqa: F401

        return
    except ImportError:
        pass
    import antenv
    from concourse import bass_utils as _bu

    mod = types.ModuleType("antenv.axon_hooks")
    _state = {}
    mod.set_axon_ntff_profile_hook = lambda h: _state.__setitem__("h", h)
    mod.get_axon_ntff_profile_hook = lambda: _state.get("h")
    sys.modules["antenv.axon_hooks"] = mod
    antenv.axon_hooks = mod
    sys.path.insert(0, "/root/.axon_site/trn_agent_boot")
    from trn_boot import _ntff_profile_via_ctypes

    mod.set_axon_ntff_profile_hook(
        _ntff_profile_via_ctypes("/opt/axon/libaxon_pjrt.so")
    )
    _bu.upload_artifacts = lambda tmpdir: f"local://{tmpdir}"


def _make_masks(h):
    kl = np.arange(P)[:, None]
    ql = np.arange(QBLK)[None, :]
    diag = [(kl <= ql - P * t).astype(np.float32) for t in range(4)]
    ones = np.ones((P, QBLK), np.float32)
    zero = np.zeros((P, QBLK), np.float32)
    tiles = diag + [zero] * 4 if h == 0 else [ones] * 4 + diag
    return np.concatenate(tiles, axis=1).astype(ml_dtypes.float8_e4m3)


def kernel(key_inputs, value_inputs, query_inputs, Wq, Wk, Wv):
    global LAST_RESULT
    import os

    key_inputs = np.asarray(key_inputs, dtype=np.float32)
    value_inputs = np.asarray(value_inputs, dtype=np.float32)
    query_inputs = np.asarray(query_inputs, dtype=np.float32)
    wq_b = (np.asarray(Wq, dtype=np.float32) * 0.125).astype(BF16)
    wk_b = np.asarray(Wk, dtype=np.float32).astype(BF16)
    wv_b = np.asarray(Wv, dtype=np.float32).astype(BF16)
    masks_np = [_make_masks(0), _make_masks(1)]
    ident_np = np.eye(P, dtype=np.float32)

    in_maps = []
    for c in range(8):
        b, h = c // 2, c % 2
        xq_c = np.concatenate(
            [query_inputs[b, q0 : q0 + QBLK] for q0 in QSTARTS[h]], axis=0
        )
        xk_c = key_inputs[b]
        xv_c = value_inputs[b]
        in_maps.append(
            {
                "xqt": np.ascontiguousarray(xq_c.T).astype(BF16),
                "xkt": np.ascontiguousarray(xk_c.T).astype(BF16),
                "xvt": np.ascontiguousarray(xv_c.T).astype(BF16),
                "wq": wq_b,
                "wk": wk_b,
                "wv": wv_b,
                "masks": masks_np[h],
                "ident": ident_np,
            }
        )

    nc = _get_nc()
    trace = bool(int(os.environ.get("KERNEL_TRACE", "0")))
    if trace:
        _ensure_ntff_hook()
    res = run_bass_kernel_spmd(
        nc,
        in_maps,
        core_ids=list(range(8)),
        trace=trace,
        tmpdir=os.environ.get("KERNEL_TRACE_DIR") or None,
    )
    LAST_RESULT = res

    out = np.empty((B, S, E), dtype=np.float32)
    for c in range(8):
        b, h = c // 2, c % 2
        z = np.asarray(res.results[c]["z"], dtype=np.float32)
        for ib, q0 in enumerate(QSTARTS[h]):
            out[b, q0 : q0 + QBLK] = z[ib * QBLK : (ib + 1) * QBLK]
    return out


# revision 16
# speedup vs baseline: 1.0965x; 1.0943x over previous
"""Causal attention head (B=4, S=4096, D=512, E=64) on 8 TRN2 NeuronCores.

Sharding: per batch b, core pair (2b, 2b+1); zig-zag query blocks
(QSTARTS) balance causal work; uniform SLOT_J keeps the program SPMD.

v3 streaming pipeline:
 - All input loads issued upfront on the sync HWDGE ring, ordered by
   first-consumer deadline (xq0, xk0, masksA, xk1, xv0, ...), so the
   ring is never blocked by compute-dependent DMAs.
 - V is projected as V^T (cheap N=512 matmuls) into vt (rows 64:80 are
   ones for the softmax denominator), then moved to [keys, E+1] chunk
   layout via SBUF->SBUF DMA-transpose issued from the *scalar* HWDGE
   ring at points where the dependency is already resolved.
 - Deferred-PV software pipeline: slot s's mask-muls + PV matmuls are
   emitted inside slot s+1's pair loop (pairs 4+), so the in-order PE
   stream never stalls the score/exp stream on V availability.  Slot 3
   uses the classic prev-pair inline pipeline for its own PVs.
 - Projection matmul groups of later segments are stuffed at pairs
   chosen to match DMA arrival times; K/Q use a [w|w] duplicated
   stationary so the partition-packed score matmuls get their
   duplicated operands for free.
 - ScalarE runs exp only (plus 4 cheap transpose-DMA issues); all
   PSUM->SBUF copies are on VectorE; z output rows are batched per slot
   and written via SWDGE (gpsimd).
 - A short burst of dummy matmuls right after the weight DMAs lifts the
   PE out of the HAM 1.2 GHz cold state before the real projections.
"""

import sys

sys.path.insert(0, "/opt/trn_rl_repo")

import numpy as np
import ml_dtypes

from concourse import bacc, mybir
from concourse import tile
from concourse.bass_utils import run_bass_kernel_spmd

BF16 = ml_dtypes.bfloat16
F32 = mybir.dt.float32
BF = mybir.dt.bfloat16

B, S, D, E = 4, 4096, 512, 64
P = 128
NQ = 2048          # queries per core
QBLK = 512         # query block
NCH = D // P       # 4 contraction chunks for projections
NKCH = S // P      # 32 key chunks
NSEG = 4           # 1024-key segments
QSTARTS = {0: [0, 1024, 2048, 3072], 1: [512, 1536, 2560, 3584]}
SLOT_J = [8, 16, 24, 32]  # uniform per-slot key-chunk counts (all cores)

_CACHE = {}
LAST_RESULT = None


def _build():
    nc = bacc.Bacc(
        "TRN2",
        target_bir_lowering=False,
        debug=False,
        enable_asserts=True,
        num_devices=8,
    )

    xqt_d = nc.declare_dram_parameter("xqt", [D, NQ], BF, isOutput=False)
    xkt_d = nc.declare_dram_parameter("xkt", [D, S], BF, isOutput=False)
    xvt_d = nc.declare_dram_parameter("xvt", [D, S], BF, isOutput=False)
    wq = nc.declare_dram_parameter("wq", [D, E], BF, isOutput=False)  # pre-scaled 1/8
    wk = nc.declare_dram_parameter("wk", [D, E], BF, isOutput=False)
    wv = nc.declare_dram_parameter("wv", [D, E], BF, isOutput=False)
    masks = nc.declare_dram_parameter("masks", [P, 8 * QBLK], BF, isOutput=False)
    ident = nc.declare_dram_parameter("ident", [P, P], F32, isOutput=False)
    zout = nc.declare_dram_parameter("z", [NQ, E], F32, isOutput=True)

    with tile.TileContext(nc) as tc:
        with (
            tc.tile_pool(name="const", bufs=1) as const,
            tc.tile_pool(name="xt", bufs=1) as xt,
            tc.tile_pool(name="proj", bufs=1) as proj,
            tc.tile_pool(name="work", bufs=16) as work,
            tc.tile_pool(name="epi", bufs=2) as epi,
            tc.tile_pool(name="psA", bufs=2, space="PSUM") as psA,
            tc.tile_pool(name="psZ", bufs=2, space="PSUM") as psZ,
            tc.tile_pool(name="psB", bufs=2, space="PSUM") as psB,
        ):
            # ---- ACT table warmup: tiny exp as early as possible ----
            warm = const.tile([1, 16], F32, tag="warm")
            warm2 = const.tile([1, 16], F32, tag="warm2")
            nc.gpsimd.memset(warm, 0.0)
            nc.scalar.activation(
                out=warm2, in_=warm, func=mybir.ActivationFunctionType.Exp
            )

            # ---- constants (duplicated wq/wk for partition-packed scores) ----
            wq_sb = const.tile([P, NCH, P], BF, tag="wq")
            wk_sb = const.tile([P, NCH, P], BF, tag="wk")
            wv_sb = const.tile([P, NCH, E], BF, tag="wv")
            for w_dram, w_sb in ((wq, wq_sb), (wk, wk_sb)):
                for half in range(2):
                    nc.sync.dma_start(
                        out=w_sb[:, :, half * E : (half + 1) * E],
                        in_=w_dram.rearrange("(c p) e -> p c e", p=P),
                    )
            nc.sync.dma_start(
                out=wv_sb[:, :, :], in_=wv.rearrange("(c p) e -> p c e", p=P)
            )
            ident_sb = const.tile([P, P], F32, tag="ident")
            nc.sync.dma_start(out=ident_sb[:, :], in_=ident[:, :])

            # ---- X tiles, per 1024-col segment ----
            xq_s = [
                xt.tile([P, NCH, 1024], BF, tag=f"xq{s}", name=f"xq_s{s}")
                for s in range(2)
            ]
            xk_s = [
                xt.tile([P, NCH, 1024], BF, tag=f"xk{s}", name=f"xk_s{s}")
                for s in range(NSEG)
            ]
            xv_s = [
                xt.tile([P, NCH, 1024], BF, tag=f"xv{s}", name=f"xv_s{s}")
                for s in range(NSEG)
            ]
            masks_sb = const.tile([P, 8 * QBLK], BF, tag="masks")

            def load_x(dst, src_d, s):
                nc.sync.dma_start(
                    out=dst[:, :, :],
                    in_=src_d[:, s * 1024 : (s + 1) * 1024].rearrange(
                        "(c p) r -> p c r", p=P
                    ),
                )

            # all input loads upfront, in first-consumer order
            load_x(xq_s[0], xqt_d, 0)
            load_x(xk_s[0], xkt_d, 0)
            nc.sync.dma_start(out=masks_sb[:, 0:2048], in_=masks[:, 0:2048])
            load_x(xk_s[1], xkt_d, 1)
            load_x(xv_s[0], xvt_d, 0)
            nc.sync.dma_start(out=masks_sb[:, 2048:4096], in_=masks[:, 2048:4096])
            nc.sync.dma_start(  # xq seg1 first half (query block 2)
                out=xq_s[1][:, :, 0:512],
                in_=xqt_d[:, 1024:1536].rearrange("(c p) r -> p c r", p=P),
            )
            load_x(xk_s[2], xkt_d, 2)
            load_x(xv_s[1], xvt_d, 1)
            nc.sync.dma_start(  # xq seg1 second half (query block 3)
                out=xq_s[1][:, :, 512:1024],
                in_=xqt_d[:, 1536:2048].rearrange("(c p) r -> p c r", p=P),
            )
            load_x(xk_s[3], xkt_d, 3)
            load_x(xv_s[2], xvt_d, 2)
            load_x(xv_s[3], xvt_d, 3)

            # ---- projection outputs ----
            qt2 = proj.tile([P, NQ], BF, tag="qt")
            kt2 = proj.tile([P, S], BF, tag="kt")
            vt = proj.tile([80, S], BF, tag="vt")       # rows 0:64 V^T, 64:80 ones
            vp = proj.tile([P, NKCH, 80], BF, tag="vp")  # [keys, chunk, E+ones]
            nc.gpsimd.memset(vt[64:80, :], 1.0)

            # ---- PE HAM warmup: dummy matmuls on the weight tiles ----
            wps = psA.tile([P, P], F32, tag="st")
            for r in range(12):
                nc.tensor.matmul(
                    wps,
                    lhsT=wq_sb[:, r % NCH, :],
                    rhs=wk_sb[:, r % NCH, :],
                    start=(r == 0),
                    stop=(r == 11),
                )
            wsink = work.tile([P, P], BF, tag="pt", name="wsink")
            nc.vector.tensor_copy(wsink, wps)

            def proj_group_kq(w_sb, x_tile, dst, seg_local_g, dst_col):
                ps = psB.tile([P, QBLK], F32, tag="pj")
                for c in range(NCH):
                    nc.tensor.matmul(
                        ps,
                        lhsT=w_sb[:, c, :],
                        rhs=x_tile[:, c, seg_local_g * QBLK : (seg_local_g + 1) * QBLK],
                        start=(c == 0),
                        stop=(c == NCH - 1),
                    )
                nc.vector.tensor_copy(dst[:, dst_col : dst_col + QBLK], ps)

            def proj_group_v(seg, seg_local_g):
                ps = psB.tile([E, QBLK], F32, tag="pj")
                for c in range(NCH):
                    nc.tensor.matmul(
                        ps,
                        lhsT=wv_sb[:, c, :],
                        rhs=xv_s[seg][:, c, seg_local_g * QBLK : (seg_local_g + 1) * QBLK],
                        start=(c == 0),
                        stop=(c == NCH - 1),
                    )
                col = seg * 1024 + seg_local_g * QBLK
                nc.vector.tensor_copy(vt[0:E, col : col + QBLK], ps)

            def transpose_v(seg):
                # scalar HWDGE ring: transfers are not queued behind the input
                # loads (sync ring FIFO); issue points are placed where the
                # vt casts are already complete so ACT never blocks.
                # vt --xbar--> vp[p][chunk][e] = V[c*128+p][e]
                nc.scalar.dma_start(
                    out=vp[:, seg * 8 : (seg + 1) * 8, :],
                    in_=vt[:, seg * 1024 : (seg + 1) * 1024],
                    transpose=True,
                )

            def k_group(seg, g):
                return lambda: proj_group_kq(
                    wk_sb, xk_s[seg], kt2, g, seg * 1024 + g * QBLK
                )

            def q_group(seg, g):
                return lambda: proj_group_kq(
                    wq_sb, xq_s[seg], qt2, g, seg * 1024 + g * QBLK
                )

            def v_group(seg, g):
                return lambda: proj_group_v(seg, g)

            # upfront projections (data arrives ~8-12us)
            proj_group_kq(wq_sb, xq_s[0], qt2, 0, 0)
            proj_group_kq(wq_sb, xq_s[0], qt2, 1, QBLK)
            proj_group_kq(wk_sb, xk_s[0], kt2, 0, 0)
            proj_group_kq(wk_sb, xk_s[0], kt2, 1, QBLK)

            # per-slot stuffing of projection groups (keyed by pair index),
            # and per-slot transpose-issue points (after exp of that pair)
            stuffing = {
                0: {2: k_group(1, 0), 3: k_group(1, 1)},
                1: {2: v_group(0, 0), 3: v_group(0, 1),
                    5: q_group(1, 0), 7: k_group(2, 0)},
                2: {0: v_group(1, 0), 1: v_group(1, 1), 2: k_group(2, 1),
                    4: q_group(1, 1), 6: k_group(3, 0), 8: k_group(3, 1),
                    10: v_group(2, 0), 11: v_group(2, 1)},
                3: {0: v_group(3, 0), 1: v_group(3, 1)},
            }
            tissue = {0: {}, 1: {4: 0}, 2: {3: 1}, 3: {0: 2, 3: 3}}

            # ---- attention ----
            def emit_epilogue(zps, ib):
                zsb = epi.tile([E + 1, QBLK], F32, tag="zsb")
                nc.vector.tensor_copy(zsb, zps)
                zf = epi.tile([P, QBLK // P, E], F32, tag="zf")
                for u in range(QBLK // P):
                    zbp = psB.tile([P, E + 1], F32, tag="pj")
                    nc.tensor.transpose(
                        zbp,
                        zsb[:, u * P : (u + 1) * P],
                        ident_sb[0 : E + 1, 0 : E + 1],
                    )
                    rc = epi.tile([P, 1], F32, tag="rc")
                    nc.vector.reciprocal(rc, zbp[:, E : E + 1])
                    nc.vector.tensor_scalar_mul(zf[:, u, :], zbp[:, 0:E], rc)
                nc.gpsimd.dma_start(
                    out=zout[ib * QBLK : (ib + 1) * QBLK, :].rearrange(
                        "(u p) e -> p u e", p=P
                    ),
                    in_=zf[:, :, :],
                )

            def mask_mul(pt, jmax, jp):
                j0 = 2 * jp
                if j0 >= jmax - 8:
                    m = j0 - (jmax - 8)
                    nc.vector.tensor_mul(
                        pt, pt, masks_sb[:, m * QBLK : (m + 2) * QBLK]
                    )

            def emit_pv(zps, jmax, pt, jp):
                for h in range(2):
                    j = 2 * jp + h
                    nc.tensor.matmul(
                        zps,
                        lhsT=vp[:, j, 0 : E + 1],
                        rhs=pt[:, h * QBLK : (h + 1) * QBLK],
                        start=(j == 0),
                        stop=(j == jmax - 1),
                        skip_group_check=True,
                    )

            # deferred records from the previous slot:
            # dict(zps, jmax, ib, pairs=[(pt, jp), ...])
            deferred = None

            for ib in range(4):
                jmax = SLOT_J[ib]
                qloc = ib * QBLK
                zps = psZ.tile([E + 1, QBLK], F32, tag="zt")
                stuff = stuffing[ib]
                tiss = tissue[ib]
                new_pairs = []
                prev = None  # slot3 inline pipeline
                drain = list(deferred["pairs"]) if deferred else []

                for jp in range(jmax // 2):
                    sps = psA.tile([P, 2 * QBLK], F32, tag="st")
                    for h in range(2):
                        j = 2 * jp + h
                        nc.tensor.matmul(
                            sps[:, h * QBLK : (h + 1) * QBLK],
                            lhsT=kt2[h * E : (h + 1) * E, j * P : (j + 1) * P],
                            rhs=qt2[h * E : (h + 1) * E, qloc : qloc + QBLK],
                            start=True,
                            stop=True,
                            tile_position=(h * E, 0),
                        )
                    pt = work.tile([P, 2 * QBLK], BF, tag="pt")
                    nc.scalar.activation(
                        out=pt, in_=sps, func=mybir.ActivationFunctionType.Exp
                    )
                    if jp in tiss:
                        transpose_v(tiss[jp])
                    if ib == 3:
                        mask_mul(pt, jmax, jp)
                        if prev is not None:
                            emit_pv(zps, jmax, *prev)
                        prev = (pt, jp)
                    if jp >= 4 and drain:
                        dpt, djp = drain.pop(0)
                        mask_mul(dpt, deferred["jmax"], djp)
                        emit_pv(deferred["zps"], deferred["jmax"], dpt, djp)
                        if not drain:
                            emit_epilogue(deferred["zps"], deferred["ib"])
                    if jp in stuff:
                        stuff[jp]()
                    if ib < 3:
                        new_pairs.append((pt, jp))

                if ib == 3:
                    emit_pv(zps, jmax, *prev)
                    # drain any leftovers (shouldn't happen: 12 fit in p4-15)
                    while drain:
                        dpt, djp = drain.pop(0)
                        mask_mul(dpt, deferred["jmax"], djp)
                        emit_pv(deferred["zps"], deferred["jmax"], dpt, djp)
                        if not drain:
                            emit_epilogue(deferred["zps"], deferred["ib"])
                    emit_epilogue(zps, ib)
                else:
                    deferred = {
                        "zps": zps,
                        "jmax": jmax,
                        "ib": ib,
                        "pairs": new_pairs,
                    }

    nc.compile()
    return nc


def _get_nc():
    if "nc" not in _CACHE:
        _CACHE["nc"] = _build()
    return _CACHE["nc"]


def _ensure_ntff_hook():
    """Install antenv.axon_hooks + NTFF profile hook if the image lacks it."""
    import types

    try:
        from antenv import axon_hooks  # noqa: F401

        return
    except ImportError:
        pass
    import antenv
    from concourse import bass_utils as _bu

    mod = types.ModuleType("antenv.axon_hooks")
    _state = {}
    mod.set_axon_ntff_profile_hook = lambda h: _state.__setitem__("h", h)
    mod.get_axon_ntff_profile_hook = lambda: _state.get("h")
    sys.modules["antenv.axon_hooks"] = mod
    antenv.axon_hooks = mod
    sys.path.insert(0, "/root/.axon_site/trn_agent_boot")
    from trn_boot import _ntff_profile_via_ctypes

    mod.set_axon_ntff_profile_hook(
        _ntff_profile_via_ctypes("/opt/axon/libaxon_pjrt.so")
    )
    _bu.upload_artifacts = lambda tmpdir: f"local://{tmpdir}"


def _make_masks(h):
    kl = np.arange(P)[:, None]
    ql = np.arange(QBLK)[None, :]
    diag = [(kl <= ql - P * t).astype(np.float32) for t in range(4)]
    ones = np.ones((P, QBLK), np.float32)
    zero = np.zeros((P, QBLK), np.float32)
    tiles = diag + [zero] * 4 if h == 0 else [ones] * 4 + diag
    return np.concatenate(tiles, axis=1).astype(BF16)


def kernel(key_inputs, value_inputs, query_inputs, Wq, Wk, Wv):
    global LAST_RESULT
    import os

    key_inputs = np.asarray(key_inputs, dtype=np.float32)
    value_inputs = np.asarray(value_inputs, dtype=np.float32)
    query_inputs = np.asarray(query_inputs, dtype=np.float32)
    wq_b = (np.asarray(Wq, dtype=np.float32) * 0.125).astype(BF16)
    wk_b = np.asarray(Wk, dtype=np.float32).astype(BF16)
    wv_b = np.asarray(Wv, dtype=np.float32).astype(BF16)
    masks_np = [_make_masks(0), _make_masks(1)]
    ident_np = np.eye(P, dtype=np.float32)

    in_maps = []
    for c in range(8):
        b, h = c // 2, c % 2
        xq_c = np.concatenate(
            [query_inputs[b, q0 : q0 + QBLK] for q0 in QSTARTS[h]], axis=0
        )
        xk_c = key_inputs[b]
        xv_c = value_inputs[b]
        in_maps.append(
            {
                "xqt": np.ascontiguousarray(xq_c.T).astype(BF16),
                "xkt": np.ascontiguousarray(xk_c.T).astype(BF16),
                "xvt": np.ascontiguousarray(xv_c.T).astype(BF16),
                "wq": wq_b,
                "wk": wk_b,
                "wv": wv_b,
                "masks": masks_np[h],
                "ident": ident_np,
            }
        )

    nc = _get_nc()
    trace = bool(int(os.environ.get("KERNEL_TRACE", "0")))
    if trace:
        _ensure_ntff_hook()
    res = run_bass_kernel_spmd(
        nc,
        in_maps,
        core_ids=list(range(8)),
        trace=trace,
        tmpdir=os.environ.get("KERNEL_TRACE_DIR") or None,
    )
    LAST_RESULT = res

    out = np.empty((B, S, E), dtype=np.float32)
    for c in range(8):
        b, h = c // 2, c % 2
        z = np.asarray(res.results[c]["z"], dtype=np.float32)
        for ib, q0 in enumerate(QSTARTS[h]):
            out[b, q0 : q0 + QBLK] = z[ib * QBLK : (ib + 1) * QBLK]
    return out
